# revision 20
# baseline (speedup 1.0000x reference)
"""Windowed correlation (cost volume) kernel for Trainium2, 8 NeuronCores.

Problem: feature1, feature2 (8, 128, 128, 256) fp32 -> out (8, 81, 128, 256),
out[b, ki*9+kj, y, x] = (1/128) * sum_c f1[b,c,y,x] * f2pad[b,c,y+ki,x+kj].

Strategy:
  - Data-parallel over batch: core i handles batch i (c=128 lands on the 128
    SBUF partitions; contraction over c runs on the TensorEngine).
  - Host marshals inputs: f1 is im2col-packed per (8y x 16x) pixel block and
    pre-cast to bf16; f2 is zero-padded (halo 4) and pre-cast to bf16 so every
    device DMA is a plain HWDGE copy.
  - Per pixel block, one bf16 matmul with lhsT = f1 block [c, 128pix] and
    rhs = the padded f2 halo block [c, 16*24=384] computes all pixel-pair
    products; the 81 useful products per pixel sit on diagonals. The rhs is
    read straight from the resident f2p with a 3-dim AP (no im2col staging).
  - The diagonal (shear) extraction is NOT done on device: no engine can
    apply a per-partition offset (BIR verifier rejects partition-crossing
    SBUF strides), and shear-gather DMAs degenerate into 18-byte
    descriptors. Instead the full [128pix, 384] slabs are stored densely
    to DRAM with line-rate DMAs (12KB contiguous per partition) and the
    host extracts the 81 diagonals per pixel with a strided view (and
    applies the 1/128 scale during the fp32 conversion).
  - PSUM->SBUF copies are split between DVE (even x0) and ACT (odd x0) so
    neither engine paces the pipeline.
  - Loads are chunked and prefetched ahead: f2 (17 row-chunks) on the
    scalar HWDGE ring, f1 (16 block-chunks) on the sync ring alongside the
    stores, keeping both rings and the DMA fabric busy end to end.

Engine plan per y0 row (pipelined; all 17 f2 chunks issued upfront on the
scalar ring, f1 prefetched 5 chunks deep on the sync ring):
  SP(sync) prefetch f1 chunk y0+5, two half-row dense stores of y0
  ACT      8 psum->stage copies (odd x0)
  PE       16 matmuls (y0) into 4 rotating PSUM banks
  DVE      8 psum->stage copies (even x0)
"""

import numpy as np

_B, _C, _H, _W = 8, 128, 128, 256
_K = 9            # kernel size (2*max_disp+1)
_ND = _K * _K     # 81 displacements
_BY, _BX = 8, 16  # pixel block (M = _BY*_BX = 128 = PE rows)
_NBY, _NBX = _H // _BY, _W // _BX        # 16 x 16 blocks
_NA, _NB = _BY + _K - 1, _BX + _K - 1    # 16 x 24 halo block
_NCOLS = _NA * _NB                       # 384 halo columns
_WIN = 288                               # 12-row halo = psum cols per half
_NAH = 12                                # halo rows per half-block matmul
_HP, _WP = _H + _K - 1, _W + _K - 1      # padded f2 dims (136, 264)
_NPS = 6                                 # rotating psum banks
_F2CH = _HP // 8                         # 17 f2 row-chunks of 8 rows

_CACHE = {}


def _build_nc():
    from contextlib import ExitStack

    import concourse.bass as bass
    import concourse.mybir as mybir

    nc = bass.Bass()
    # f1 comes in host-packed: [c, y0, x0*128 + ry*16 + rx] bf16
    f1 = nc.dram_tensor(
        "f1", [_C, _NBY, _NBX * 128], mybir.dt.bfloat16, kind="ExternalInput"
    )
    f2 = nc.dram_tensor("f2", [_C, _HP, _WP], mybir.dt.bfloat16, kind="ExternalInput")
    # windowed psum slabs: [y0, m, x0, 288] (host extracts the diagonals).
    # Each pixel block is computed as TWO M=64 matmuls (pixel rows ry<4 and
    # ry>=4) against 12-row halos, so each psum bank is a uniformly-useful
    # [128, 288] tile: full-width engine copies, long store runs, and 25%
    # fewer HBM write bytes than the naive 384-wide slab.
    out = nc.dram_tensor(
        "out", [_NBY, 128, _NBX, _WIN], mybir.dt.bfloat16, kind="ExternalOutput"
    )

    rows = _NBY
    with ExitStack() as ctx:
        f1blk = ctx.enter_context(
            nc.sbuf_tensor([_C, _NBY * _NBX * 128], mybir.dt.bfloat16)
        )
        f2p = ctx.enter_context(nc.sbuf_tensor([_C, _HP * _WP], mybir.dt.bfloat16))
        stage = [
            ctx.enter_context(
                nc.sbuf_tensor(f"stg{i}", [_C, _NBX * _WIN], mybir.dt.bfloat16)
            )
            for i in range(3)
        ]
        psum = [
            ctx.enter_context(
                nc.psum_tensor(f"ps{i}", [128, _WIN], mybir.dt.float32)
            )
            for i in range(_NPS)
        ]
        s_f1 = ctx.enter_context(nc.semaphore(name="s_f1"))    # +16 per f1 chunk
        s_f2 = ctx.enter_context(nc.semaphore(name="s_f2"))    # +16 per f2 chunk
        s_pe = ctx.enter_context(nc.semaphore(name="s_pe"))    # +1 per matmul
        s_dve = ctx.enter_context(nc.semaphore(name="s_dve"))  # +1 per DVE copy
        s_sc = ctx.enter_context(nc.semaphore(name="s_sc"))    # +1 per ACT copy
        s_st = ctx.enter_context(nc.semaphore(name="s_st"))    # +16 per half store
        blk = ctx.enter_context(nc.Block())

        def load_f2_chunk(eng, j):
            src = bass.AP(
                tensor=f2,
                offset=j * 8 * _WP,
                ap=[[_HP * _WP, _C], [1, 8 * _WP]],
            )
            eng.dma_start(f2p[:, j * 8 * _WP : (j + 1) * 8 * _WP], src).then_inc(
                s_f2, 16
            )

        def load_f1_chunk(eng, j):
            src = bass.AP(
                tensor=f1,
                offset=j * _NBX * 128,
                ap=[[_NBY * _NBX * 128, _C], [1, _NBX * 128]],
            )
            eng.dma_start(
                f1blk[:, j * _NBX * 128 : (j + 1) * _NBX * 128], src
            ).then_inc(s_f1, 16)

        def copy_sem_wait(eng, n):
            # WAR on psum bank n % 4: its previous user n-4 must be copied out.
            # Copies alternate DVE (even) / ACT (odd); n-4 has n's parity.
            if n >= _NPS:
                m = n - _NPS
                eng.wait_ge(s_dve if m % 2 == 0 else s_sc, m // 2 + 1)

        @blk.sync
        def _(sync):
            for j in range(5):
                load_f1_chunk(sync, j)
            for r in range(rows):
                if r + 5 < rows:
                    load_f1_chunk(sync, r + 5)
                # half-row stores once the 8 covering stage copies are done
                for h in range(2):
                    sync.wait_ge(s_dve, r * 8 + 4 * (h + 1))
                    sync.wait_ge(s_sc, r * 8 + 4 * (h + 1))
                    half = _NBX * _WIN // 2
                    dst = bass.AP(
                        tensor=out,
                        offset=r * 128 * _NBX * _WIN + h * half,
                        ap=[[_NBX * _WIN, _C], [1, half]],
                    )
                    sync.dma_start(
                        dst, stage[r % 3][:, h * half : (h + 1) * half]
                    ).then_inc(s_st, 16)
            sync.wait_ge(s_st, rows * 32)

        @blk.scalar
        def _(scalar):
            # f2 loads ride the scalar HWDGE ring (nothing queues behind
            # them, so issue all 17 upfront); f1/stores ride sync's.
            for j in range(_F2CH):
                load_f2_chunk(scalar, j)
            for r in range(rows):
                # WAR: stores of r-3 read this stage buffer
                if r >= 3:
                    scalar.wait_ge(s_st, (r - 2) * 32)
                for x0 in range(1, _NBX, 2):
                    n = r * _NBX + x0
                    scalar.wait_ge(s_pe, n + 1)
                    st = stage[r % 3][:, x0 * _WIN : (x0 + 1) * _WIN]
                    nc.scalar.activation(
                        st, psum[n % _NPS][:, :], mybir.ActivationFunctionType.Copy
                    ).then_inc(s_sc, 1)

        @blk.tensor
        def _(tensor):
            for r in range(rows):
                tensor.wait_ge(s_f1, (r + 1) * 16)
                # matmuls read f2p rows [8r, 8r+16) = chunks r, r+1
                tensor.wait_ge(s_f2, (r + 2) * 16)
                for x0 in range(_NBX):
                    n = r * _NBX + x0
                    copy_sem_wait(tensor, n)
                    base = (r * _NBX + x0) * 128
                    # half A: pixels ry<4, halo rows [8r, 8r+12)
                    rhs_a = bass.AP(
                        tensor=f2p,
                        offset=r * _BY * _WP + x0 * _BX,
                        ap=[[_HP * _WP, _C], [_WP, _NAH], [1, _NB]],
                    )
                    nc.tensor.matmul(
                        psum[n % _NPS][0:64, :],
                        f1blk[:, base : base + 64],
                        rhs_a,
                        start=True,
                        stop=True,
                    )
                    # half B: pixels ry>=4, halo rows [8r+4, 8r+16)
                    rhs_b = bass.AP(
                        tensor=f2p,
                        offset=(r * _BY + 4) * _WP + x0 * _BX,
                        ap=[[_HP * _WP, _C], [_WP, _NAH], [1, _NB]],
                    )
                    nc.tensor.matmul(
                        psum[n % _NPS][64:128, :],
                        f1blk[:, base + 64 : base + 128],
                        rhs_b,
                        start=True,
                        stop=True,
                    ).then_inc(s_pe, 1)

        @blk.vector
        def _(vector):
            for r in range(rows):
                # WAR: stores of r-3 read this stage buffer
                if r >= 3:
                    vector.wait_ge(s_st, (r - 2) * 32)
                for x0 in range(0, _NBX, 2):
                    n = r * _NBX + x0
                    vector.wait_ge(s_pe, n + 1)
                    st = stage[r % 3][:, x0 * _WIN : (x0 + 1) * _WIN]
                    nc.vector.tensor_copy(st, psum[n % _NPS][:, :]).then_inc(
                        s_dve, 1
                    )

    return nc


def _pack_f1(f1_core: np.ndarray) -> np.ndarray:
    """[c, h, w] fp32 -> [c, y0, x0*128 + ry*16 + rx] bf16."""
    import ml_dtypes

    v = f1_core.reshape(_C, _NBY, _BY, _NBX, _BX)
    v = v.transpose(0, 1, 3, 2, 4)  # c, y0, x0, ry, rx
    return np.ascontiguousarray(v.reshape(_C, _NBY, _NBX * 128)).astype(
        ml_dtypes.bfloat16
    )


def _pack_f2(f2_core: np.ndarray) -> np.ndarray:
    """[c, h, w] fp32 -> zero-padded [c, 136, 264] bf16."""
    import ml_dtypes

    f2p = np.zeros((_C, _HP, _WP), dtype=ml_dtypes.bfloat16)
    f2p[:, 4 : 4 + _H, 4 : 4 + _W] = f2_core.astype(ml_dtypes.bfloat16)
    return f2p


def _prep_in_maps(f1: np.ndarray, f2: np.ndarray) -> list:
    return [{"f1": _pack_f1(f1[i]), "f2": _pack_f2(f2[i])} for i in range(_B)]


def _unpack_out(raw: np.ndarray) -> np.ndarray:
    """Windowed slab [y0, m, x0, 288] bf16 -> [81, h, w] fp32.

    Pixel (y0, ry, x0, rx) displacement (ki, kj) lives at m = ry*16+rx,
    col = (ry%4)*24 + rx + ki*24 + kj (the two half-block matmuls use
    12-row halos starting at ry-half*4, so the 96-col offset for ry>=4
    never exists on device). Applies the deferred 1/c scale.
    """
    import ml_dtypes
    from numpy.lib.stride_tricks import as_strided

    u = np.ascontiguousarray(raw).view(np.uint16).reshape(-1)
    e = u.itemsize
    ms = _NBX * _WIN  # m stride (elements)
    g = as_strided(
        u,
        shape=(_NBY, 2, 4, _NBX, _BX, _K, _K),
        strides=(
            128 * ms * e,         # y0
            64 * ms * e,          # ry-high (fresh 12-row halo, col resets)
            (16 * ms + _NB) * e,  # ry-low: m += 16, col += 24
            _WIN * e,             # x0
            (ms + 1) * e,         # rx: m += 1, col += 1
            _NB * e,              # ki: col += 24
            1 * e,                # kj
        ),
    )
    # -> [ki, kj, y0, ryh, ryl, x0, rx] -> (81, 128, 256)
    dense = np.ascontiguousarray(g.transpose(5, 6, 0, 1, 2, 3, 4)).reshape(
        _ND, _H, _W
    )
    return dense.view(ml_dtypes.bfloat16).astype(np.float32) * (1.0 / _C)


def kernel(feature1: np.ndarray, feature2: np.ndarray) -> np.ndarray:
    from concourse.bass_utils import run_bass_kernel_spmd

    if "nc" not in _CACHE:
        _CACHE["nc"] = _build_nc()
    nc = _CACHE["nc"]

    f1 = np.ascontiguousarray(np.asarray(feature1), dtype=np.float32)
    f2 = np.ascontiguousarray(np.asarray(feature2), dtype=np.float32)
    in_maps = _prep_in_maps(f1, f2)
    res = run_bass_kernel_spmd(nc, in_maps, core_ids=list(range(_B)))
    out = np.stack([_unpack_out(res.results[i]["out"]) for i in range(_B)], axis=0)
    return out
